# revision 23
# baseline (speedup 1.0000x reference)
"""AutoCorrelation block (Autoformer-style) on 8 trn2 NeuronCores.

Single launch, fully on-device, data-parallel over batch (one batch per
core). The axon tunnel moves only ~30-70 MB/s, so the design minimizes
host<->device bytes (~66 MB total vs ~560 MB for the two-launch +
host-FFT version):

  inputs:  xT bf16 [512, 4096] per core (4 MB), a 256-column weight
           SHARD per core (256 KB) that is AllGathered on-device over
           NeuronLink, biases.
  outputs: int8 [4096, 512] with per-row fp32 scales (2 MB per core;
           donated zero output buffers ship over the tunnel too, so
           halving the output dtype pays twice).

On-device pipeline:
  1. Q/K/V projections (PE matmuls, bf16 in / f32 psum, bias fused).
  2. mean autocorrelation WITHOUT FFT: for each 128-column tile of K,
     the Gram tile H[p, t] = <k_{s0+p}, q_t> is written to DRAM doubled
     ([H|H]); a strided "shear" read (partition step 2L+1) realigns
     diagonals so mean_corr[tau] = sum_p H2[p, s0+p+tau] accumulates
     with plain vector adds + a 1/512-ones matmul reduction.
  3. top-8 delays via DVE max8/max_index, softmax via ACT exp.
  4. circular gather: V is stored doubled in DRAM [128, 8192] x4; each
     delay is read from SBUF into a gpsimd register and becomes a
     dynamic-offset DMA window read, accumulated as
     agg += w_i * window (DVE scalar_tensor_tensor).
  5. output projection in [l, d] orientation (bias via a contraction-1
     ones matmul), per-row int8 quantization with round-to-nearest.

Host does only: bf16 casts/transposes (threaded), weight sharding, and
the int8 dequant into the final f32 array. No FFT, no top-k, no rolls.

Built with the Tile framework (auto-scheduling/semaphores); a post-pass
splits multi-wait instructions because this walrus build only accepts
one embedded sync-wait per instruction.
"""

import sys
from concurrent.futures import ThreadPoolExecutor

import numpy as np

for p in ("/opt/trn_rl_repo",):
    if p not in sys.path:
        sys.path.insert(0, p)

import ml_dtypes

import concourse.bass as bass
import concourse.mybir as mybir
from concourse.bass_utils import run_bass_kernel_spmd
from concourse.tile import TileContext

import bass_rust

B, L, D = 8, 4096, 512
TOP_K = 8  # int(1.0 * log(4096)) = 8
N_CORES = 8
F32 = mybir.dt.float32
BF16 = mybir.dt.bfloat16
U32 = mybir.dt.uint32
MS = bass.MemorySpace

NC = D // 128   # 4 contraction tiles
NL = L // 512   # 8 free-dim chunks of 512
LC = 512

_compiled = {}
_SPLIT_WAITS = True  # walrus needs it; CoreSim's race detector does not cope


def _build():
    nc = bass.Bass()
    xT = nc.dram_tensor("xT", [D, L], BF16, kind="ExternalInput")
    # per-core weight shard: [wqkv cols 192*b..  | wo cols 64*b..] packed
    wsh = nc.dram_tensor("wsh", [D, 256], BF16, kind="ExternalInput")
    wsh_b = nc.dram_tensor("wsh_b", [D, 256], BF16, kind="Internal")
    wall = nc.dram_tensor(
        "wall", [8, D, 256], BF16, kind="Internal", addr_space="Shared")
    bias = nc.dram_tensor("bias", [D, 4], F32, kind="ExternalInput")
    oq = nc.dram_tensor("oq", [L, D], mybir.dt.int8, kind="ExternalOutput")
    osc = nc.dram_tensor("osc", [L, 1], F32, kind="ExternalOutput")

    # one V2 buffer per 128-row tile so the dynamic-gather AP offset is
    # exactly the snapped register value (avoids a per-gather computed
    # offset register at lowering time)
    v2s = [
        nc.dram_tensor(f"v2_{i}", [128, 2 * L], F32, kind="Internal")
        for i in range(NC)
    ]
    h2 = [
        nc.dram_tensor(f"h2_{i}", [128, 2 * L], F32, kind="Internal")
        for i in range(2)
    ]

    AluOp = mybir.AluOpType
    Act = mybir.ActivationFunctionType

    gsem_ctx = nc.semaphore("gsem")
    gsem = gsem_ctx.__enter__()
    gcnt = 0

    with TileContext(nc) as tc:
        with (
            tc.tile_pool(name="persist", bufs=1) as p_per,
            tc.tile_pool(name="psum", bufs=6, space=MS.PSUM) as p_ps,
            tc.tile_pool(name="psmall", bufs=1, space=MS.PSUM) as p_ps2,
        ):
            # persistent across phases: top-k results
            vals8 = p_per.tile([1, 8], F32, name="vals8", tag="vals8")
            idx8 = p_per.tile([1, 8], U32, name="idx8", tag="idx8")
            w8 = p_per.tile([1, 8], F32, name="w8", tag="w8")
            wbc = p_per.tile([128, 8], F32, name="wbc", tag="wbc")

            # Q/K live through phases 1-2 only
            pqk_ctx = tc.tile_pool(name="qk", bufs=1)
            p_qk = pqk_ctx.__enter__()
            Qt = [p_qk.tile([128, L], BF16, name=f"q{i}", tag=f"q{i}") for i in range(NC)]
            Kt = [p_qk.tile([128, L], BF16, name=f"k{i}", tag=f"k{i}") for i in range(NC)]

            # ---------------- phase 1: load + QKV projections ----------
            with tc.tile_pool(name="ph1", bufs=1) as p1, \
                 tc.tile_pool(name="vst", bufs=2) as pv:
                xt = [p1.tile([128, L], BF16, name=f"x{i}", tag=f"x{i}") for i in range(NC)]
                wt = [p1.tile([128, 3 * D], BF16, name=f"w{i}", tag=f"w{i}") for i in range(NC)]
                bt = [p1.tile([128, 4], F32, name=f"b{i}", tag=f"b{i}") for i in range(NC)]
                for c in range(NC):
                    r = slice(c * 128, (c + 1) * 128)
                    nc.sync.dma_start(xt[c][:], xT[r, :])
                    nc.sync.dma_start(bt[c][:], bias[r, :])
                # all-gather the 8 weight shards (2MB over NeuronLink
                # instead of 2MB per core over the axon tunnel)
                nc.gpsimd.dma_start(wsh_b[:, :], wsh[:, :])
                nc.gpsimd.collective_compute(
                    "AllGather",
                    AluOp.bypass,
                    replica_groups=[list(range(N_CORES))],
                    ins=[wsh_b[:, :].opt()],
                    outs=[wall[:, :, :].opt()],
                )
                for c in range(NC):
                    r = slice(c * 128, (c + 1) * 128)
                    for rk in range(N_CORES):
                        nc.sync.dma_start(
                            wt[c][:, rk * 192:(rk + 1) * 192],
                            wall[rk, r, 0:192])

                # Q and K -> bf16 SBUF tiles (bias fused in ACT copy)
                for which, dest in ((0, Qt), (1, Kt)):
                    for mt in range(NC):
                        for lc in range(NL):
                            ps = p_ps.tile([128, LC], F32, name="mm", tag="mm")
                            for ct in range(NC):
                                nc.tensor.matmul(
                                    ps[:],
                                    wt[ct][:, which * D + mt * 128:
                                           which * D + (mt + 1) * 128],
                                    xt[ct][:, lc * LC:(lc + 1) * LC],
                                    start=(ct == 0),
                                    stop=(ct == NC - 1),
                                )
                            nc.scalar.activation(
                                dest[mt][:, lc * LC:(lc + 1) * LC], ps[:],
                                Act.Identity, bias=bt[mt][:, which:which + 1],
                            )
                # V -> f32, doubled into DRAM
                for mt in range(NC):
                    vrow = pv.tile([128, L], F32, name="vrow", tag="vrow")
                    for lc in range(NL):
                        ps = p_ps.tile([128, LC], F32, name="mm", tag="mm")
                        for ct in range(NC):
                            nc.tensor.matmul(
                                ps[:],
                                wt[ct][:, 2 * D + mt * 128:2 * D + (mt + 1) * 128],
                                xt[ct][:, lc * LC:(lc + 1) * LC],
                                start=(ct == 0),
                                stop=(ct == NC - 1),
                            )
                        nc.scalar.activation(
                            vrow[:, lc * LC:(lc + 1) * LC], ps[:],
                            Act.Identity, bias=bt[mt][:, 2:3],
                        )
                    nc.sync.dma_start(v2s[mt][:, 0:L], vrow[:])
                    nc.sync.dma_start(v2s[mt][:, L:2 * L], vrow[:])

            # ------- phase 2: Gram tiles + shear diag accumulation ------
            with tc.tile_pool(name="ph2", bufs=2) as p2, \
                 tc.tile_pool(name="ph2acc", bufs=1) as p2a:
                acc = p2a.tile([128, L], F32, name="acc", tag="acc")
                ones = p2a.tile([128, 1], F32, name="ones", tag="ones")
                corr = p2a.tile([1, L], F32, name="corr", tag="corr")
                nc.vector.memset(acc[:], 0.0)
                nc.vector.memset(ones[:], 1.0 / D)

                for st in range(L // 128):
                    s0 = st * 128
                    hrow = p2.tile([128, L], F32, name="hrow", tag="hrow")
                    for tck in range(NL):
                        ps = p_ps.tile([128, LC], F32, name="mm", tag="mm")
                        for ct in range(NC):
                            nc.tensor.matmul(
                                ps[:],
                                Kt[ct][:, s0:s0 + 128],
                                Qt[ct][:, tck * LC:(tck + 1) * LC],
                                start=(ct == 0),
                                stop=(ct == NC - 1),
                            )
                        nc.vector.tensor_copy(
                            hrow[:, tck * LC:(tck + 1) * LC], ps[:])
                    hbuf = h2[st % 2]
                    nc.sync.dma_start(hbuf[:, 0:L], hrow[:])
                    nc.sync.dma_start(hbuf[:, L:2 * L], hrow[:])
                    # shear read: tmp[p, tau] = hbuf[p, s0 + p + tau]
                    shear = bass_rust.AP(
                        hbuf[:, :].tensor, s0, [[2 * L + 1, 128], [1, L]])
                    tmp = p2.tile([128, L], F32, name="tmp", tag="tmp")
                    nc.sync.dma_start(tmp[:], shear)
                    nc.vector.tensor_add(acc[:], acc[:], tmp[:])

                # corr[tau] = sum_p acc[p, tau] / D  (ones matmul)
                for tck in range(NL):
                    pc = p_ps2.tile([1, LC], F32, name="corrps", tag="corrps")
                    nc.tensor.matmul(
                        pc[:], ones[:], acc[:, tck * LC:(tck + 1) * LC],
                        start=True, stop=True,
                    )
                    nc.vector.tensor_copy(corr[:, tck * LC:(tck + 1) * LC], pc[:])

                # ---- top-8 + softmax (still inside ph2 scope: corr) ----
                nc.vector.max(out=vals8[:], in_=corr[:])
                nc.vector.max_index(out=idx8[:], in_max=vals8[:], in_values=corr[:])
                nm = p2a.tile([1, 1], F32, name="nm", tag="nm")
                e8 = p2a.tile([1, 8], F32, name="e8", tag="e8")
                ssum = p2a.tile([1, 1], F32, name="ssum", tag="ssum")
                rsum = p2a.tile([1, 1], F32, name="rsum", tag="rsum")
                ones1 = p2a.tile([1, 128], F32, name="ones1", tag="ones1")
                nc.vector.tensor_scalar(
                    out=nm[:], in0=vals8[:, 0:1], scalar1=-1.0, scalar2=None,
                    op0=AluOp.mult)
                nc.scalar.activation(e8[:], vals8[:], Act.Exp, bias=nm[0:1, 0:1])
                nc.vector.reduce_sum(ssum[:], e8[:], axis=mybir.AxisListType.X)
                nc.vector.reciprocal(rsum[:], ssum[:])
                nc.vector.tensor_scalar(
                    out=w8[:], in0=e8[:], scalar1=rsum[0:1, 0:1], scalar2=None,
                    op0=AluOp.mult)
                # broadcast w8 across partitions: ones1^T @ w8
                nc.vector.memset(ones1[:], 1.0)
                pb = p_ps2.tile([128, 8], F32, name="wbcps", tag="wbcps")
                nc.tensor.matmul(pb[:], ones1[:], w8[:], start=True, stop=True)
                nc.vector.tensor_copy(wbc[:], pb[:])

            pqk_ctx.__exit__(None, None, None)

            # ------- phase 3: gather + aggregate + output projection ----
            with tc.tile_pool(name="ph3", bufs=1) as p3, \
                 tc.tile_pool(name="gat", bufs=2) as pg, \
                 tc.tile_pool(name="ost", bufs=4) as po:
                agg = [p3.tile([128, L], F32, name=f"agg{i}", tag=f"agg{i}") for i in range(NC)]
                for mt in range(NC):
                    nc.vector.memset(agg[mt][:], 0.0)
                for i in range(TOP_K):
                    for mt in range(NC):
                        gt = pg.tile([128, L], F32, name="g", tag="g")
                        gcnt += 16
                        with tc.tile_critical():
                            with nc.gpsimd.register(f"dv{i}_{mt}") as reg:
                                nc.gpsimd.reg_load(reg, idx8[0:1, i:i + 1])
                                dv = nc.gpsimd.snap(
                                    reg, donate=True, min_val=0, max_val=L - 1)
                                nc.gpsimd.dma_start(
                                    gt[:],
                                    v2s[mt][:, bass.ds(dv, L)],
                                ).then_inc(gsem, 16)
                                nc.gpsimd.wait_ge(gsem, gcnt)
                        nc.vector.scalar_tensor_tensor(
                            out=agg[mt][:], in0=gt[:],
                            scalar=wbc[:, i:i + 1], in1=agg[mt][:],
                            op0=AluOp.mult, op1=AluOp.add,
                        )

                # Output projection in [l, d] orientation:
                #   out[l, m] = sum_c agg[c, l] * WoT[c, m] + bo[m]
                # (bo folded in as a contraction-1 ones matmul), then int8
                # row quantization: oq[l, :] = round(out[l, :] / osc[l]),
                # osc[l] = absmax(out[l, :]) / 127. Halves the shipped
                # output bytes (donated zero buffers ship too).
                wob = [p3.tile([128, D], BF16, name=f"wob{i}", tag=f"wob{i}") for i in range(NC)]
                wof = [p3.tile([128, D], F32, name=f"wof{i}", tag=f"wof{i}") for i in range(NC)]
                bo_row = p3.tile([1, D], F32, name="bo_row", tag="bo_row")
                ones_r = p3.tile([1, 128], F32, name="ones_r", tag="ones_r")
                for c in range(NC):
                    r = slice(c * 128, (c + 1) * 128)
                    for rk in range(N_CORES):
                        nc.sync.dma_start(
                            wob[c][:, rk * 64:(rk + 1) * 64],
                            wall[rk, r, 192:256])
                    nc.vector.tensor_copy(wof[c][:], wob[c][:])
                nc.sync.dma_start(bo_row[:], bias[:, 3:4])
                nc.vector.memset(ones_r[:], 1.0)
                for lt in range(L // 128):
                    l0 = lt * 128
                    ps = p_ps.tile([128, D], F32, name="mm", tag="mm")
                    for ct in range(NC):
                        nc.tensor.matmul(
                            ps[:],
                            agg[ct][:, l0:l0 + 128],
                            wof[ct][:],
                            start=(ct == 0),
                            stop=False,
                        )
                    nc.tensor.matmul(
                        ps[:], ones_r[:], bo_row[:], start=False, stop=True)
                    am = po.tile([128, 1], F32, name="am", tag="am")
                    rc = po.tile([128, 1], F32, name="rc", tag="rc")
                    t2 = po.tile([128, D], F32, name="t2", tag="t2")
                    sg = po.tile([128, D], F32, name="sg", tag="sg")
                    qi = po.tile([128, D], mybir.dt.int8, name="qi", tag="qi")
                    nc.vector.tensor_reduce(
                        am[:], ps[:], axis=mybir.AxisListType.X,
                        op=AluOp.max, apply_absolute_value=True)
                    nc.vector.tensor_scalar_max(am[:], am[:], 1e-30)
                    nc.vector.reciprocal(rc[:], am[:])
                    nc.vector.tensor_scalar(
                        out=t2[:], in0=ps[:], scalar1=rc[:, 0:1],
                        scalar2=127.0, op0=AluOp.mult, op1=AluOp.mult)
                    nc.scalar.activation(sg[:], t2[:], Act.Sign)
                    nc.vector.scalar_tensor_tensor(
                        out=qi[:], in0=sg[:], scalar=0.5, in1=t2[:],
                        op0=AluOp.mult, op1=AluOp.add)
                    nc.vector.tensor_scalar(
                        out=am[:], in0=am[:], scalar1=1.0 / 127.0,
                        scalar2=None, op0=AluOp.mult)
                    nc.sync.dma_start(oq[l0:l0 + 128, :], qi[:])
                    nc.sync.dma_start(osc[l0:l0 + 128, :], am[:])

    gsem_ctx.__exit__(None, None, None)
    if _SPLIT_WAITS:
        _split_multi_waits(nc)
    return nc


def _split_multi_waits(nc):
    """This walrus build rejects any instruction with more than one
    embedded sync-wait; hoist extras onto standalone NOPs just before the
    instruction (same engine, same basic block)."""
    for blk in nc.m.functions[0].blocks:
        insts = list(blk.instructions)
        out, n = [], 0
        for inst in insts:
            si = inst.sync_info
            if si is not None and len(si.on_wait) > 1:
                waits = list(si.on_wait)
                for w in waits[:-1]:
                    nop = mybir.InstNoOp(
                        name=f"wsplit_{blk.name}_{n}", text_hint="wait_split")
                    n += 1
                    nop.engine = inst.engine
                    nop.sync_info = mybir.SyncInfo(on_wait=[w], on_update=[])
                    out.append(nop)
                inst.sync_info = mybir.SyncInfo(
                    on_wait=[waits[-1]], on_update=list(si.on_update))
            out.append(inst)
        if len(out) != len(insts):
            blk.instructions = out


def _get_nc():
    if "nc" not in _compiled:
        _compiled["nc"] = _build()
    return _compiled["nc"]


def _prep_inputs(hidden_states, Wq, bq, Wk, bk, Wv, bv, Wo, bo):
    bf = ml_dtypes.bfloat16
    wqkv = np.concatenate([Wq.T, Wk.T, Wv.T], axis=1).astype(bf)
    woT = Wo.T.astype(bf)
    bias = np.stack([bq, bk, bv, bo], axis=1).astype(np.float32)
    hb = hidden_states.astype(bf)
    with ThreadPoolExecutor(B) as ex:
        xs = list(ex.map(lambda b: np.ascontiguousarray(hb[b].T), range(B)))
    return [
        {
            "xT": xs[b],
            "wsh": np.ascontiguousarray(np.concatenate(
                [wqkv[:, b * 192:(b + 1) * 192],
                 woT[:, b * 64:(b + 1) * 64]], axis=1)),
            "bias": bias,
        }
        for b in range(B)
    ]


def kernel(hidden_states, Wq, bq, Wk, bk, Wv, bv, Wo, bo):
    hidden_states = np.asarray(hidden_states, np.float32)
    Wq, Wk, Wv, Wo = (np.asarray(a, np.float32) for a in (Wq, Wk, Wv, Wo))
    bq, bk, bv, bo = (np.asarray(a, np.float32) for a in (bq, bk, bv, bo))

    nc = _get_nc()
    in_maps = _prep_inputs(hidden_states, Wq, bq, Wk, bk, Wv, bv, Wo, bo)
    res = run_bass_kernel_spmd(nc, in_maps, list(range(N_CORES))).results

    out = np.empty((B, L, D), np.float32)

    def _deq(b):
        np.multiply(res[b]["oq"], res[b]["osc"], out=out[b], casting="unsafe")

    with ThreadPoolExecutor(B) as ex:
        list(ex.map(_deq, range(B)))
    return out


# revision 24
# speedup vs baseline: 1.1582x; 1.1582x over previous
"""AutoCorrelation block (Autoformer-style) on 8 trn2 NeuronCores.

Single launch, fully on-device, data-parallel over batch (one batch per
core). The axon tunnel moves only ~30-70 MB/s, so the design minimizes
host<->device bytes (~66 MB total vs ~560 MB for the two-launch +
host-FFT version):

  inputs:  xT bf16 [512, 4096] per core (4 MB), a 256-column weight
           SHARD per core (256 KB) that is AllGathered on-device over
           NeuronLink, biases.
  outputs: int8 [4096, 512] with per-row fp32 scales (2 MB per core;
           donated zero output buffers ship over the tunnel too, so
           halving the output dtype pays twice).

On-device pipeline:
  1. Q/K/V projections (PE matmuls, bf16 in / f32 psum, bias fused).
  2. mean autocorrelation WITHOUT FFT: for each 128-column tile of K,
     the Gram tile H[p, t] = <k_{s0+p}, q_t> is written to DRAM doubled
     ([H|H]); a strided "shear" read (partition step 2L+1) realigns
     diagonals so mean_corr[tau] = sum_p H2[p, s0+p+tau] accumulates
     with plain vector adds + a 1/512-ones matmul reduction.
  3. top-8 delays via DVE max8/max_index, softmax via ACT exp.
  4. circular gather: V is stored doubled in DRAM [128, 8192] x4; each
     delay is read from SBUF into a gpsimd register and becomes a
     dynamic-offset DMA window read, accumulated as
     agg += w_i * window (DVE scalar_tensor_tensor).
  5. output projection in [l, d] orientation (bias via a contraction-1
     ones matmul), per-row int8 quantization with round-to-nearest.

Host does only: bf16 casts/transposes (threaded), weight sharding, and
the int8 dequant into the final f32 array. No FFT, no top-k, no rolls.

Built with the Tile framework (auto-scheduling/semaphores); a post-pass
splits multi-wait instructions because this walrus build only accepts
one embedded sync-wait per instruction.
"""

import sys
from concurrent.futures import ThreadPoolExecutor

import numpy as np

for p in ("/opt/trn_rl_repo",):
    if p not in sys.path:
        sys.path.insert(0, p)

import ml_dtypes

import concourse.bass as bass
import concourse.mybir as mybir
from concourse.bass_utils import run_bass_kernel_spmd
from concourse.tile import TileContext

import bass_rust

B, L, D = 8, 4096, 512
TOP_K = 8  # int(1.0 * log(4096)) = 8
N_CORES = 8
F32 = mybir.dt.float32
BF16 = mybir.dt.bfloat16
U32 = mybir.dt.uint32
MS = bass.MemorySpace

NC = D // 128   # 4 contraction tiles
NL = L // 512   # 8 free-dim chunks of 512
LC = 512

_compiled = {}
_SPLIT_WAITS = True  # walrus needs it; CoreSim's race detector does not cope


def _build():
    nc = bass.Bass()
    xT = nc.dram_tensor("xT", [D, L], BF16, kind="ExternalInput")
    # per-core weight shard: [wqkv cols 192*b..  | wo cols 64*b..] packed
    wsh = nc.dram_tensor("wsh", [D, 256], BF16, kind="ExternalInput")
    wsh_b = nc.dram_tensor("wsh_b", [D, 256], BF16, kind="Internal")
    wall = nc.dram_tensor(
        "wall", [8, D, 256], BF16, kind="Internal", addr_space="Shared")
    bias = nc.dram_tensor("bias", [D, 4], F32, kind="ExternalInput")
    oq = nc.dram_tensor("oq", [L, D], mybir.dt.int8, kind="ExternalOutput")
    osc = nc.dram_tensor("osc", [L, 1], F32, kind="ExternalOutput")

    # one V2 buffer per 128-row tile so the dynamic-gather AP offset is
    # exactly the snapped register value (avoids a per-gather computed
    # offset register at lowering time)
    v2s = [
        nc.dram_tensor(f"v2_{i}", [128, 2 * L], F32, kind="Internal")
        for i in range(NC)
    ]
    h2 = [
        nc.dram_tensor(f"h2_{i}", [128, 2 * L], F32, kind="Internal")
        for i in range(2)
    ]

    AluOp = mybir.AluOpType
    Act = mybir.ActivationFunctionType

    gsem_ctx = nc.semaphore("gsem")
    gsem = gsem_ctx.__enter__()
    gcnt = 0

    with TileContext(nc) as tc:
        with (
            tc.tile_pool(name="persist", bufs=1) as p_per,
            tc.tile_pool(name="psum", bufs=6, space=MS.PSUM) as p_ps,
            tc.tile_pool(name="psmall", bufs=1, space=MS.PSUM) as p_ps2,
        ):
            # persistent across phases: top-k results
            vals8 = p_per.tile([1, 8], F32, name="vals8", tag="vals8")
            idx8 = p_per.tile([1, 8], U32, name="idx8", tag="idx8")
            w8 = p_per.tile([1, 8], F32, name="w8", tag="w8")
            wbc = p_per.tile([128, 8], F32, name="wbc", tag="wbc")

            # Q/K live through phases 1-2 only
            pqk_ctx = tc.tile_pool(name="qk", bufs=1)
            p_qk = pqk_ctx.__enter__()
            Qt = [p_qk.tile([128, L], BF16, name=f"q{i}", tag=f"q{i}") for i in range(NC)]
            Kt = [p_qk.tile([128, L], BF16, name=f"k{i}", tag=f"k{i}") for i in range(NC)]

            # ---------------- phase 1: load + QKV projections ----------
            with tc.tile_pool(name="ph1", bufs=1) as p1, \
                 tc.tile_pool(name="vst", bufs=2) as pv:
                xt = [p1.tile([128, L], BF16, name=f"x{i}", tag=f"x{i}") for i in range(NC)]
                wt = [p1.tile([128, 3 * D], BF16, name=f"w{i}", tag=f"w{i}") for i in range(NC)]
                bt = [p1.tile([128, 4], F32, name=f"b{i}", tag=f"b{i}") for i in range(NC)]
                for c in range(NC):
                    r = slice(c * 128, (c + 1) * 128)
                    nc.sync.dma_start(xt[c][:], xT[r, :])
                    nc.sync.dma_start(bt[c][:], bias[r, :])
                # all-gather the 8 weight shards (2MB over NeuronLink
                # instead of 2MB per core over the axon tunnel)
                nc.gpsimd.dma_start(wsh_b[:, :], wsh[:, :])
                nc.gpsimd.collective_compute(
                    "AllGather",
                    AluOp.bypass,
                    replica_groups=[list(range(N_CORES))],
                    ins=[wsh_b[:, :].opt()],
                    outs=[wall[:, :, :].opt()],
                )
                for c in range(NC):
                    r = slice(c * 128, (c + 1) * 128)
                    for rk in range(N_CORES):
                        nc.sync.dma_start(
                            wt[c][:, rk * 192:(rk + 1) * 192],
                            wall[rk, r, 0:192])

                # Q and K -> bf16 SBUF tiles (bias fused in ACT copy)
                for which, dest in ((0, Qt), (1, Kt)):
                    for mt in range(NC):
                        for lc in range(NL):
                            ps = p_ps.tile([128, LC], F32, name="mm", tag="mm")
                            for ct in range(NC):
                                nc.tensor.matmul(
                                    ps[:],
                                    wt[ct][:, which * D + mt * 128:
                                           which * D + (mt + 1) * 128],
                                    xt[ct][:, lc * LC:(lc + 1) * LC],
                                    start=(ct == 0),
                                    stop=(ct == NC - 1),
                                )
                            nc.scalar.activation(
                                dest[mt][:, lc * LC:(lc + 1) * LC], ps[:],
                                Act.Identity, bias=bt[mt][:, which:which + 1],
                            )
                # V -> f32, doubled into DRAM
                for mt in range(NC):
                    vrow = pv.tile([128, L], F32, name="vrow", tag="vrow")
                    for lc in range(NL):
                        ps = p_ps.tile([128, LC], F32, name="mm", tag="mm")
                        for ct in range(NC):
                            nc.tensor.matmul(
                                ps[:],
                                wt[ct][:, 2 * D + mt * 128:2 * D + (mt + 1) * 128],
                                xt[ct][:, lc * LC:(lc + 1) * LC],
                                start=(ct == 0),
                                stop=(ct == NC - 1),
                            )
                        nc.scalar.activation(
                            vrow[:, lc * LC:(lc + 1) * LC], ps[:],
                            Act.Identity, bias=bt[mt][:, 2:3],
                        )
                    nc.sync.dma_start(v2s[mt][:, 0:L], vrow[:])
                    nc.sync.dma_start(v2s[mt][:, L:2 * L], vrow[:])

            # ------- phase 2: Gram tiles + shear diag accumulation ------
            with tc.tile_pool(name="ph2", bufs=2) as p2, \
                 tc.tile_pool(name="ph2acc", bufs=1) as p2a:
                acc = p2a.tile([128, L], F32, name="acc", tag="acc")
                ones = p2a.tile([128, 1], F32, name="ones", tag="ones")
                corr = p2a.tile([1, L], F32, name="corr", tag="corr")
                nc.vector.memset(acc[:], 0.0)
                nc.vector.memset(ones[:], 1.0 / D)

                for st in range(L // 128):
                    s0 = st * 128
                    hrow = p2.tile([128, L], F32, name="hrow", tag="hrow")
                    for tck in range(NL):
                        ps = p_ps.tile([128, LC], F32, name="mm", tag="mm")
                        for ct in range(NC):
                            nc.tensor.matmul(
                                ps[:],
                                Kt[ct][:, s0:s0 + 128],
                                Qt[ct][:, tck * LC:(tck + 1) * LC],
                                start=(ct == 0),
                                stop=(ct == NC - 1),
                            )
                        nc.vector.tensor_copy(
                            hrow[:, tck * LC:(tck + 1) * LC], ps[:])
                    hbuf = h2[st % 2]
                    nc.sync.dma_start(hbuf[:, 0:L], hrow[:])
                    nc.sync.dma_start(hbuf[:, L:2 * L], hrow[:])
                    # shear read: tmp[p, tau] = hbuf[p, s0 + p + tau]
                    shear = bass_rust.AP(
                        hbuf[:, :].tensor, s0, [[2 * L + 1, 128], [1, L]])
                    tmp = p2.tile([128, L], F32, name="tmp", tag="tmp")
                    nc.sync.dma_start(tmp[:], shear)
                    nc.vector.tensor_add(acc[:], acc[:], tmp[:])

                # corr[tau] = sum_p acc[p, tau] / D  (ones matmul)
                for tck in range(NL):
                    pc = p_ps2.tile([1, LC], F32, name="corrps", tag="corrps")
                    nc.tensor.matmul(
                        pc[:], ones[:], acc[:, tck * LC:(tck + 1) * LC],
                        start=True, stop=True,
                    )
                    nc.vector.tensor_copy(corr[:, tck * LC:(tck + 1) * LC], pc[:])

                # ---- top-8 + softmax (still inside ph2 scope: corr) ----
                nc.vector.max(out=vals8[:], in_=corr[:])
                nc.vector.max_index(out=idx8[:], in_max=vals8[:], in_values=corr[:])
                nm = p2a.tile([1, 1], F32, name="nm", tag="nm")
                e8 = p2a.tile([1, 8], F32, name="e8", tag="e8")
                ssum = p2a.tile([1, 1], F32, name="ssum", tag="ssum")
                rsum = p2a.tile([1, 1], F32, name="rsum", tag="rsum")
                ones1 = p2a.tile([1, 128], F32, name="ones1", tag="ones1")
                nc.vector.tensor_scalar(
                    out=nm[:], in0=vals8[:, 0:1], scalar1=-1.0, scalar2=None,
                    op0=AluOp.mult)
                nc.scalar.activation(e8[:], vals8[:], Act.Exp, bias=nm[0:1, 0:1])
                nc.vector.reduce_sum(ssum[:], e8[:], axis=mybir.AxisListType.X)
                nc.vector.reciprocal(rsum[:], ssum[:])
                nc.vector.tensor_scalar(
                    out=w8[:], in0=e8[:], scalar1=rsum[0:1, 0:1], scalar2=None,
                    op0=AluOp.mult)
                # broadcast w8 across partitions: ones1^T @ w8
                nc.vector.memset(ones1[:], 1.0)
                pb = p_ps2.tile([128, 8], F32, name="wbcps", tag="wbcps")
                nc.tensor.matmul(pb[:], ones1[:], w8[:], start=True, stop=True)
                nc.vector.tensor_copy(wbc[:], pb[:])

            pqk_ctx.__exit__(None, None, None)

            # ------- phase 3: gather + aggregate + output projection ----
            with tc.tile_pool(name="ph3", bufs=1) as p3, \
                 tc.tile_pool(name="gat", bufs=2) as pg, \
                 tc.tile_pool(name="ost", bufs=4) as po:
                agg = [p3.tile([128, L], F32, name=f"agg{i}", tag=f"agg{i}") for i in range(NC)]
                for mt in range(NC):
                    nc.vector.memset(agg[mt][:], 0.0)
                for i in range(TOP_K):
                    for mt in range(NC):
                        gt = pg.tile([128, L], F32, name="g", tag="g")
                        gcnt += 16
                        with tc.tile_critical():
                            with nc.gpsimd.register(f"dv{i}_{mt}") as reg:
                                nc.gpsimd.reg_load(reg, idx8[0:1, i:i + 1])
                                dv = nc.gpsimd.snap(
                                    reg, donate=True, min_val=0, max_val=L - 1)
                                nc.gpsimd.dma_start(
                                    gt[:],
                                    v2s[mt][:, bass.ds(dv, L)],
                                ).then_inc(gsem, 16)
                                nc.gpsimd.wait_ge(gsem, gcnt)
                        nc.vector.scalar_tensor_tensor(
                            out=agg[mt][:], in0=gt[:],
                            scalar=wbc[:, i:i + 1], in1=agg[mt][:],
                            op0=AluOp.mult, op1=AluOp.add,
                        )

                # Output projection in [l, d] orientation:
                #   out[l, m] = sum_c agg[c, l] * WoT[c, m] + bo[m]
                # (bo folded in as a contraction-1 ones matmul), then int8
                # row quantization: oq[l, :] = round(out[l, :] / osc[l]),
                # osc[l] = absmax(out[l, :]) / 127. Halves the shipped
                # output bytes (donated zero buffers ship too).
                wob = [p3.tile([128, D], BF16, name=f"wob{i}", tag=f"wob{i}") for i in range(NC)]
                wof = [p3.tile([128, D], F32, name=f"wof{i}", tag=f"wof{i}") for i in range(NC)]
                bo_row = p3.tile([1, D], F32, name="bo_row", tag="bo_row")
                ones_r = p3.tile([1, 128], F32, name="ones_r", tag="ones_r")
                for c in range(NC):
                    r = slice(c * 128, (c + 1) * 128)
                    for rk in range(N_CORES):
                        nc.sync.dma_start(
                            wob[c][:, rk * 64:(rk + 1) * 64],
                            wall[rk, r, 192:256])
                    nc.vector.tensor_copy(wof[c][:], wob[c][:])
                nc.sync.dma_start(bo_row[:], bias[:, 3:4])
                nc.vector.memset(ones_r[:], 1.0)
                for lt in range(L // 128):
                    l0 = lt * 128
                    ps = p_ps.tile([128, D], F32, name="mm", tag="mm")
                    for ct in range(NC):
                        nc.tensor.matmul(
                            ps[:],
                            agg[ct][:, l0:l0 + 128],
                            wof[ct][:],
                            start=(ct == 0),
                            stop=False,
                        )
                    nc.tensor.matmul(
                        ps[:], ones_r[:], bo_row[:], start=False, stop=True)
                    am = po.tile([128, 1], F32, name="am", tag="am")
                    rc = po.tile([128, 1], F32, name="rc", tag="rc")
                    t2 = po.tile([128, D], F32, name="t2", tag="t2")
                    sg = po.tile([128, D], F32, name="sg", tag="sg")
                    qi = po.tile([128, D], mybir.dt.int8, name="qi", tag="qi")
                    nc.vector.tensor_reduce(
                        am[:], ps[:], axis=mybir.AxisListType.X,
                        op=AluOp.max, apply_absolute_value=True)
                    nc.vector.tensor_scalar_max(am[:], am[:], 1e-30)
                    nc.vector.reciprocal(rc[:], am[:])
                    nc.vector.tensor_scalar(
                        out=t2[:], in0=ps[:], scalar1=rc[:, 0:1],
                        scalar2=127.0, op0=AluOp.mult, op1=AluOp.mult)
                    nc.scalar.activation(sg[:], t2[:], Act.Sign)
                    nc.vector.scalar_tensor_tensor(
                        out=qi[:], in0=sg[:], scalar=0.5, in1=t2[:],
                        op0=AluOp.mult, op1=AluOp.add)
                    nc.vector.tensor_scalar(
                        out=am[:], in0=am[:], scalar1=1.0 / 127.0,
                        scalar2=None, op0=AluOp.mult)
                    nc.sync.dma_start(oq[l0:l0 + 128, :], qi[:])
                    nc.sync.dma_start(osc[l0:l0 + 128, :], am[:])

    gsem_ctx.__exit__(None, None, None)
    if _SPLIT_WAITS:
        _split_multi_waits(nc)
    return nc


def _split_multi_waits(nc):
    """This walrus build rejects any instruction with more than one
    embedded sync-wait; hoist extras onto standalone NOPs just before the
    instruction (same engine, same basic block)."""
    for blk in nc.m.functions[0].blocks:
        insts = list(blk.instructions)
        out, n = [], 0
        for inst in insts:
            si = inst.sync_info
            if si is not None and len(si.on_wait) > 1:
                waits = list(si.on_wait)
                for w in waits[:-1]:
                    nop = mybir.InstNoOp(
                        name=f"wsplit_{blk.name}_{n}", text_hint="wait_split")
                    n += 1
                    nop.engine = inst.engine
                    nop.sync_info = mybir.SyncInfo(on_wait=[w], on_update=[])
                    out.append(nop)
                inst.sync_info = mybir.SyncInfo(
                    on_wait=[waits[-1]], on_update=list(si.on_update))
            out.append(inst)
        if len(out) != len(insts):
            blk.instructions = out


def _get_nc():
    if "nc" not in _compiled:
        _compiled["nc"] = _build()
    return _compiled["nc"]


def _prep_inputs(hidden_states, Wq, bq, Wk, bk, Wv, bv, Wo, bo):
    bf = ml_dtypes.bfloat16
    wqkv = np.concatenate([Wq.T, Wk.T, Wv.T], axis=1).astype(bf)
    woT = Wo.T.astype(bf)
    bias = np.stack([bq, bk, bv, bo], axis=1).astype(np.float32)
    with ThreadPoolExecutor(B) as ex:
        xs = list(ex.map(
            lambda b: np.ascontiguousarray(hidden_states[b].astype(bf).T),
            range(B)))
    return [
        {
            "xT": xs[b],
            "wsh": np.ascontiguousarray(np.concatenate(
                [wqkv[:, b * 192:(b + 1) * 192],
                 woT[:, b * 64:(b + 1) * 64]], axis=1)),
            "bias": bias,
        }
        for b in range(B)
    ]


def kernel(hidden_states, Wq, bq, Wk, bk, Wv, bv, Wo, bo):
    hidden_states = np.asarray(hidden_states, np.float32)
    Wq, Wk, Wv, Wo = (np.asarray(a, np.float32) for a in (Wq, Wk, Wv, Wo))
    bq, bk, bv, bo = (np.asarray(a, np.float32) for a in (bq, bk, bv, bo))

    nc = _get_nc()
    in_maps = _prep_inputs(hidden_states, Wq, bq, Wk, bk, Wv, bv, Wo, bo)
    res = run_bass_kernel_spmd(nc, in_maps, list(range(N_CORES))).results

    out = np.empty((B, L, D), np.float32)

    def _deq(b):
        np.multiply(res[b]["oq"], res[b]["osc"], out=out[b], casting="unsafe")

    with ThreadPoolExecutor(B) as ex:
        list(ex.map(_deq, range(B)))
    return out
